# revision 3
# baseline (speedup 1.0000x reference)
"""DenseKAN forward as a single fused matmul on TRN2.

Math: the reference uses a uniform knot grid (spacing h=0.4 on
[-2.2, 2.2]), so the Cox-de Boor bases are shifted copies of the
cardinal cubic B-spline:

    B_j(x) = Q(u - j),  u = (x + 2.2)/0.4 = 2.5x + 5.5,  u in [3, 8)
    Q(s)   = (1/6) * sum_{m=0..4} (-1)^m C(4,m) relu(s-m)^3

For u in [3, 8), relu(u-n)^3 == 0 for n >= 8, so the 8 bases are exact
linear combinations of the 8 features r_n = relu(u-n)^3, n = 0..7.
Folding the binomial coefficients, the per-dim scale factor, and the
bias (via partition of unity, sum_j B_j == 1) into the weights on the
host turns the whole layer into:

    out[b,o] = sum_{n,i} r_n(x[b,i]) * W2[n,i,o] + sum_i silu(x[b,i]) * SF[i,o]

i.e. one (B, 2304) @ (2304, 256) matmul with on-chip features.
Batch is sharded across the 8 cores (128 rows each); weights are
replicated.
"""

import numpy as np

import concourse.bass as bass
import concourse.mybir as mybir
import concourse.tile as tile
from concourse import bacc
from concourse.bass_utils import run_bass_kernel_spmd
from concourse.masks import make_identity

BATCH = 1024
IN = 256
UNITS = 256
GK = 8  # grid_size + spline_order = number of spline bases
NF = GK + 1  # + silu feature block
K = IN * NF  # 2304 contraction rows
N_CORES = 8
BS = BATCH // N_CORES  # 128 batch rows per core
KT = K // 128  # 18 K-tiles
W_CHUNK = 3  # K-tiles per weight DMA

FP32 = mybir.dt.float32


def _build():
    nc = bacc.Bacc("TRN2", target_bir_lowering=False, debug=False,
                   num_devices=N_CORES)
    x_d = nc.dram_tensor("x", [BS, IN], FP32, kind="ExternalInput").ap()
    w_d = nc.dram_tensor("w2", [K, UNITS], FP32, kind="ExternalInput").ap()
    o_d = nc.dram_tensor("out", [BS, UNITS], FP32, kind="ExternalOutput").ap()

    with tile.TileContext(nc) as tc:
        with (
            tc.tile_pool(name="const", bufs=1) as cpool,
            tc.tile_pool(name="sq", bufs=2) as sqpool,
            tc.tile_pool(name="psum", bufs=1, space="PSUM") as ppool,
        ):
            # weights: (128, kt, o); DRAM row 128*kt + p
            w2 = cpool.tile([128, KT, UNITS], FP32)
            n_chunks = KT // W_CHUNK
            for c in range(n_chunks):
                src = w_d[c * W_CHUNK * 128:(c + 1) * W_CHUNK * 128, :]
                nc.sync.dma_start(
                    w2[:, c * W_CHUNK:(c + 1) * W_CHUNK, :],
                    src.rearrange("(k p) o -> p k o", p=128),
                )

            xsb = cpool.tile([BS, IN], FP32)
            nc.sync.dma_start(xsb[:], x_d[:])

            ident = cpool.tile([128, 128], FP32)
            make_identity(nc, ident[:])

            btile = cpool.tile([128, GK], FP32)
            for n in range(GK):
                nc.gpsimd.memset(btile[:, n:n + 1], float(5.5 - n))

            # x^T: partition = input-dim (mod 128), free = [itile, b]
            xTp = ppool.tile([128, BS * 2], FP32)
            nc.tensor.transpose(xTp[:, 0:BS], xsb[:, 0:128], ident[:])
            nc.tensor.transpose(xTp[:, BS:2 * BS], xsb[:, 128:256], ident[:])
            xT = cpool.tile([128, BS * 2], FP32)
            nc.vector.tensor_copy(xT[:], xTp[:])

            # features: blocks n=0..7 are relu(2.5x + 5.5 - n)^3, block 8 silu
            T = cpool.tile([128, NF * BS * 2], FP32)
            opsum = ppool.tile([BS, UNITS], FP32)

            # silu block first so PE can start while relu blocks compute
            nc.scalar.activation(T[:, GK * 256:(GK + 1) * 256], xT[:],
                                 mybir.ActivationFunctionType.Silu)
            nc.tensor.matmul(opsum[:], T[:, 2048 + 0:2048 + 128],
                             w2[:, 16, :], start=True, stop=False)
            nc.tensor.matmul(opsum[:], T[:, 2048 + 128:2048 + 256],
                             w2[:, 17, :], start=False, stop=False)

            for n in range(GK):
                blk = T[:, n * 256:(n + 1) * 256]
                nc.scalar.activation(blk, xT[:],
                                     mybir.ActivationFunctionType.Relu,
                                     bias=btile[:, n:n + 1], scale=2.5)
                sq = sqpool.tile([128, 256], FP32)
                nc.vector.tensor_mul(sq[:], blk, blk)
                nc.vector.tensor_mul(blk, sq[:], blk)
                for h in range(2):
                    k = 2 * n + h
                    nc.tensor.matmul(opsum[:],
                                     T[:, k * 128:(k + 1) * 128],
                                     w2[:, k, :], start=False,
                                     stop=(k == 2 * GK - 1))

            osb = cpool.tile([BS, UNITS], FP32)
            nc.vector.tensor_copy(osb[:], opsum[:])
            nc.sync.dma_start(o_d[:], osb[:])

    nc.compile()
    return nc


def _fold_weights(spline_kernel, scale_factor, bias):
    """(IN, GK, UNITS) spline kernel -> (K, UNITS) folded weight matrix."""
    sk = spline_kernel.astype(np.float64)
    sf = scale_factor.astype(np.float64)
    b = bias.astype(np.float64)
    # W[i,j,o] = sk*sf + bias/IN  (bias folded via sum_j B_j == 1)
    W = sk * sf[:, None, :] + b[None, None, :] / IN
    # B_j = sum_m (-1)^m C(4,m)/6 * r_{j+m}  ->  A[j, n] coeff of r_n
    comb = np.array([1.0, -4.0, 6.0, -4.0, 1.0]) / 6.0
    A = np.zeros((GK, GK))
    for j in range(GK):
        for m in range(5):
            if j + m < GK:
                A[j, j + m] = comb[m]
    # W2[n,i,o] = sum_j A[j,n] W[i,j,o]
    W2 = np.einsum("jn,ijo->nio", A, W)
    Wfull = np.concatenate([W2, sf[None, :, :]], axis=0)  # (NF, IN, UNITS)
    return np.ascontiguousarray(Wfull.reshape(K, UNITS).astype(np.float32))


_cache = {}


def kernel(x, spline_kernel, scale_factor, bias):
    if "nc" not in _cache:
        _cache["nc"] = _build()
    nc = _cache["nc"]

    w2 = _fold_weights(spline_kernel, scale_factor, bias)
    x = np.ascontiguousarray(x, dtype=np.float32)
    in_maps = [
        {"x": x[c * BS:(c + 1) * BS], "w2": w2} for c in range(N_CORES)
    ]
    res = run_bass_kernel_spmd(nc, in_maps, list(range(N_CORES)))
    out = np.concatenate([res.results[c]["out"] for c in range(N_CORES)], axis=0)
    return out.astype(np.float32)


# revision 6
# speedup vs baseline: 1.4716x; 1.4716x over previous
"""DenseKAN forward as a single fused matmul on TRN2.

Math: the reference uses a uniform knot grid (spacing h=0.4 on
[-2.2, 2.2]), so the Cox-de Boor bases are shifted copies of the
cardinal cubic B-spline with u = 2.5x + 5.5 in [3, 8):

    B_j(x) = Q(u - j),   Q(s) = (1/6) sum_m (-1)^m C(4,m) relu(s-m)^3

Using Q's symmetry Q(s) = Q(4-s), each basis is expanded from the side
that keeps the truncated-power features small (bounded by 64 instead of
512, which keeps the binomial cancellation mild enough for reduced-
precision matmul):

    blocks 0..3:  f_n = relu((n+4) - u)^3      (right-side powers)
    blocks 4..7:  f_n = relu(u - n)^3          (left-side powers)
    block  8:     silu(x)

    B_0 = f_0/6                 B_7 = f_7/6
    B_1 = (f_1 - 4 f_0)/6       B_6 = (f_6 - 4 f_7)/6
    B_2 = (f_2 - 4 f_1 + 6 f_0)/6    etc.

Folding those coefficients, the per-dim scale factor, and the bias (via
partition of unity, sum_j B_j == 1) into the weights on the host turns
the whole layer into out = F(x) @ W2 with F computed on-chip via
8 ACT relu ops + 2 DVE squares/cubes per block. Batch is sharded
across the 8 cores (128 rows each); weights are replicated.
"""

import numpy as np

import concourse.bass as bass
import concourse.mybir as mybir
import concourse.tile as tile
from concourse import bacc
from concourse.bass_utils import run_bass_kernel_spmd
from concourse.masks import make_identity

BATCH = 1024
IN = 256
UNITS = 256
GK = 8  # number of spline bases per input dim
NF = GK + 1  # + silu feature block
K = IN * NF  # 2304 contraction rows
N_CORES = 8
BS = BATCH // N_CORES  # 128 batch rows per core
KT = K // 128  # 18 K-tiles
W_CHUNKS = 6

FP32 = mybir.dt.float32
MM_DT = mybir.dt.float32r  # matmul compute dtype (fp32 bit layout)

_cache = {}


def _build():
    nc = bacc.Bacc("TRN2", target_bir_lowering=False, debug=False,
                   num_devices=N_CORES)
    x_d = nc.dram_tensor("x", [BS, IN], FP32, kind="ExternalInput").ap()
    # host pre-swizzled: w2[p, k, o] = W2_flat[128*k + p, o]
    w_d = nc.dram_tensor("w2", [128, KT, UNITS], MM_DT,
                         kind="ExternalInput").ap()
    o_d = nc.dram_tensor("out", [BS, UNITS], FP32, kind="ExternalOutput").ap()

    with tile.TileContext(nc) as tc:
        with (
            tc.tile_pool(name="const", bufs=1) as cpool,
            tc.tile_pool(name="sq", bufs=2) as sqpool,
            tc.tile_pool(name="psum", bufs=1, space="PSUM") as ppool,
        ):
            # x first: the feature pipeline needs it, and it is tiny
            xsb = cpool.tile([BS, IN], FP32)
            nc.sync.dma_start(xsb[:], x_d[:])

            # weights stream in behind x, one contiguous chunk at a time
            w2 = cpool.tile([128, KT, UNITS], MM_DT)
            kt_c = KT // W_CHUNKS
            for c in range(W_CHUNKS):
                nc.sync.dma_start(w2[:, c * kt_c:(c + 1) * kt_c, :],
                                  w_d[:, c * kt_c:(c + 1) * kt_c, :])

            ident = cpool.tile([128, 128], FP32)
            make_identity(nc, ident[:])

            # bias column j holds j - 1.5 (shared by blocks n and 7-n)
            btile = cpool.tile([128, 4], FP32)
            for j in range(4):
                nc.gpsimd.memset(btile[:, j:j + 1], float(j - 1.5))

            # x^T in PSUM: partition = input-dim (mod 128), free = [itile, b]
            xTp = ppool.tile([128, BS * 2], FP32)
            nc.tensor.transpose(xTp[:, 0:BS], xsb[:, 0:128], ident[:])
            nc.tensor.transpose(xTp[:, BS:2 * BS], xsb[:, 128:256], ident[:])

            # features
            T = cpool.tile([128, NF * 256], MM_DT)
            opsum = ppool.tile([BS, UNITS], FP32)

            # weight k-tile order (host side matches): silu pair first,
            # then feature blocks in compute order
            nc.scalar.activation(T[:, GK * 256:(GK + 1) * 256], xTp[:],
                                 mybir.ActivationFunctionType.Silu)
            nc.tensor.matmul(opsum[:], T[:, 2048:2176],
                             w2[:, 0, :],
                             start=True, stop=False)
            nc.tensor.matmul(opsum[:], T[:, 2176:2304],
                             w2[:, 1, :],
                             start=False, stop=False)

            for n in range(GK):
                blk = T[:, n * 256:(n + 1) * 256]
                scale = -2.5 if n < 4 else 2.5
                bj = n if n < 4 else 7 - n
                nc.scalar.activation(blk, xTp[:],
                                     mybir.ActivationFunctionType.Relu,
                                     bias=btile[:, bj:bj + 1], scale=scale)
                sq = sqpool.tile([128, 256], FP32)
                nc.vector.tensor_mul(sq[:], blk, blk)
                nc.vector.tensor_mul(blk, sq[:], blk)
                for h in range(2):
                    k = 2 * n + h
                    nc.tensor.matmul(opsum[:],
                                     T[:, k * 128:(k + 1) * 128],
                                     w2[:, 2 + k, :],
                                     start=False, stop=(k == 2 * GK - 1))

            osb = cpool.tile([BS, UNITS], FP32)
            nc.vector.tensor_copy(osb[:], opsum[:])
            nc.sync.dma_start(o_d[:], osb[:])

    nc.compile()
    return nc


def _fold_weights(spline_kernel, scale_factor, bias):
    """-> (128, KT, UNITS) swizzled folded weights, w2[p,k,o]=W2[128k+p,o]."""
    sk = spline_kernel.astype(np.float64)
    sf = scale_factor.astype(np.float64)
    b = bias.astype(np.float64)
    # W[i,j,o] = sk*sf + bias/IN  (bias folded via sum_j B_j == 1)
    W = sk * sf[:, None, :] + b[None, None, :] / IN
    comb = np.array([1.0, -4.0, 6.0, -4.0, 1.0]) / 6.0
    # A[j, n] = coefficient of feature-block n in basis j
    A = np.zeros((GK, GK))
    for j in range(4):  # right-side: B_j = sum_m comb[m] * q_{(j+4)-m}
        for m in range(j + 1):
            A[j, j - m] = comb[m]
    for j in range(4, GK):  # left-side: B_j = sum_m comb[m] * r_{j+m}
        for m in range(GK - j):
            A[j, j + m] = comb[m]
    W2 = np.einsum("jn,ijo->nio", A, W)  # (GK, IN, UNITS)
    Wfull = np.concatenate([sf[None, :, :], W2], axis=0)  # (NF, IN, UNITS)
    flat = Wfull.reshape(K, UNITS)
    # swizzle to [p, k, o]
    sw = flat.reshape(KT, 128, UNITS).transpose(1, 0, 2)
    return np.ascontiguousarray(sw.astype(np.float32))


def kernel(x, spline_kernel, scale_factor, bias):
    if "nc" not in _cache:
        _cache["nc"] = _build()
    nc = _cache["nc"]

    w2 = _fold_weights(spline_kernel, scale_factor, bias)
    x = np.ascontiguousarray(x, dtype=np.float32)
    in_maps = [
        {"x": x[c * BS:(c + 1) * BS], "w2": w2} for c in range(N_CORES)
    ]
    res = run_bass_kernel_spmd(nc, in_maps, list(range(N_CORES)))
    out = np.concatenate([res.results[c]["out"] for c in range(N_CORES)],
                         axis=0)
    return out.astype(np.float32)
